# revision 1
# baseline (speedup 1.0000x reference)
"""Trainium2 Bass kernel for nn_DeepRMSAFeatureExtractor.

Strategy (8 NeuronCores, SPMD, collective-free):
  The prior sharded design paid ~75us of collective machinery (runtime
  barrier absorbing launch skew + a 29us AllGather) to save ~14us of DMA.
  This version replicates the FULL computation on every core -- zero
  collectives, so each core's span is skew-free -- and instead attacks
  the memory roofline with precision:
    - W0's Hm block (16384x128) and alpha block (16384x128) are cast to
      fp8e4m3 on host (pure relayout+cast; no model math on host).
      Measured end-to-end rel err 0.53% vs the 2e-2 gate.
    - The Hm block collapses: x_hm = kron(w[b,:], WH) so
      x_hm @ W0hm = w @ G with G[i,h] = sum_f WH[f] W0hm[(i,f),h].
      G^T is built with 128 N=1 matmuls that depend only on the W0hm DMA
      (not on attention), then one transpose + one matmul finish the block.
    - The alpha block needs NO transposes: attention tiles are stored
      [128 i-part, (b,j) free] and W0a keeps its natural
      reshape(128, 16384) = [i, (j,h)] layout, so k-tile j contracts
      over i directly with rhs = alpha_all[:, j::128].
    - W0 chunk DMAs are gate-chained (tiny SP-engine sem-wait DMAs
      between posts) so chunks complete sequentially and the matmuls
      pipeline behind the DMA stream.
  Every core computes the full [16,128] output; core 0's is returned.
"""

import sys

sys.path.insert(0, "/opt/trn_rl_repo")

import numpy as np

import concourse.bass as bass  # noqa: F401  (registers AP machinery)
import concourse.bacc as bacc
import concourse.mybir as mybir
import concourse.tile as tile
from concourse import bass_utils

F32 = mybir.dt.float32
BF16 = mybir.dt.bfloat16
F8 = mybir.dt.float8e4

NCORES = 8
BATCH = 16
N = 128          # nodes == HID
HID = 128
M_EDGES = 256
EF = 17
KP = 5

# offsets into the flat `inputs` row (length 86721)
OFF_SD = 0
OFF_SLOT = 256
OFF_SPEC = 261
OFF_LF = 321
OFF_BET = 4673
OFF_ADJ = 4801

MISC_ROWS = 321                  # source_dest 256 + slots 5 + c_band 30 + l_band 30
MISC_PAD = 384                   # 3 k-tiles of 128
HM_OFF = 0
MISC_OFF = N * HID               # 16384
ALPHA_OFF = MISC_OFF + MISC_ROWS  # 16705

NCHUNK = 4                       # w0hm / w0a split into 4 chunks of 32 k-tiles


def shard_inputs(inputs, WH, WE, a_attn, W0, b0, Wr, br):
    """Host-side prep: slicing / transposition / padding / dtype cast only."""
    f = np.float32
    bf = mybir.dt.np(BF16)
    f8 = mybir.dt.np(F8)
    X = np.ascontiguousarray(np.asarray(inputs, dtype=f))
    WH = np.asarray(WH, dtype=f)
    WE = np.asarray(WE, dtype=f)
    a_attn = np.asarray(a_attn, dtype=f)
    W0 = np.asarray(W0, dtype=f)
    b0 = np.asarray(b0, dtype=f)
    Wr = np.asarray(Wr, dtype=f)
    br = np.asarray(br, dtype=f)

    bet = X[:, OFF_BET:OFF_BET + N]                                        # [16,128]
    adj = X[:, OFF_ADJ:OFF_ADJ + N * N].reshape(BATCH, N, N)
    # lfT[m, e*16 + b] = link_features[b, m, e]
    lfT = np.ascontiguousarray(
        X[:, OFF_LF:OFF_LF + M_EDGES * EF]
        .reshape(BATCH, M_EDGES, EF)
        .transpose(1, 2, 0)
        .reshape(M_EDGES, BATCH * EF)
    ).astype(bf)                                                           # [256,272]

    spec = X[:, OFF_SPEC:OFF_SPEC + KP * 12].reshape(BATCH, KP, 2, 6)
    xm = np.zeros((BATCH, MISC_PAD), dtype=f)
    xm[:, 0:256] = X[:, OFF_SD:OFF_SD + 256]
    xm[:, 256:261] = X[:, OFF_SLOT:OFF_SLOT + 5]
    xm[:, 261:291] = spec[:, :, 0, :].reshape(BATCH, 30)
    xm[:, 291:321] = spec[:, :, 1, :].reshape(BATCH, 30)
    # xmt[p, t*16 + b] = xm[b, t*128 + p]
    xmt = np.ascontiguousarray(
        xm.T.reshape(3, 128, BATCH).transpose(1, 0, 2).reshape(128, 3 * BATCH)
    ).astype(bf)

    w0m = np.zeros((MISC_PAD, HID), dtype=f)
    w0m[:MISC_ROWS] = W0[MISC_OFF:MISC_OFF + MISC_ROWS]
    # w0m_dev[p, t*128 + h] = w0m[t*128 + p, h]
    w0m_dev = np.ascontiguousarray(
        w0m.reshape(3, 128, HID).transpose(1, 0, 2).reshape(128, 3 * HID)
    ).astype(bf)

    # w0hm8[f, i*128 + h] = W0[i*128 + f, h]  (k-tile i: [128f, 128h])
    w0hm8 = np.ascontiguousarray(
        W0[HM_OFF:HM_OFF + N * HID].reshape(N, HID, HID)
        .transpose(1, 0, 2).reshape(128, N * HID)
    ).astype(f8)
    # w0a8[i, j*128 + h] = W0[ALPHA_OFF + i*128 + j, h]  (k-tile j: [128i, 128h])
    w0a8 = np.ascontiguousarray(
        W0[ALPHA_OFF:ALPHA_OFF + N * N].reshape(128, N * HID)
    ).astype(f8)

    common = {
        "w0hm8": w0hm8,                                                    # [128,16384] fp8
        "w0a8": w0a8,                                                      # [128,16384] fp8
        "w0m": w0m_dev,                                                    # [128,384] bf16
        "xmt": xmt,                                                        # [128,48] bf16
        "adj8": np.ascontiguousarray(
            adj.transpose(1, 0, 2).reshape(128, BATCH * N)).astype(f8),    # [128,2048] fp8
        "bet16": np.ascontiguousarray(bet).astype(bf),                     # [16,128] bf16
        "bett": np.ascontiguousarray(bet.T),                               # [128,16] f32
        "lft": np.ascontiguousarray(lfT),                                  # [256,272] bf16
        "wht": np.ascontiguousarray(WH.reshape(1, HID).T),                 # [128,1] f32
        "whcol": np.ascontiguousarray(WH.reshape(1, HID).T).astype(bf),    # [128,1] bf16
        "a12": np.ascontiguousarray(
            np.stack([a_attn[:HID, 0], a_attn[HID:2 * HID, 0]], axis=1)),  # [128,2]
        "a3": np.ascontiguousarray(a_attn[2 * HID:, 0:1]),                 # [128,1]
        "wet": np.ascontiguousarray(WE.T),                                 # [128,17]
        "we0": np.ascontiguousarray(WE[:, 0:1]),                           # [17,1]
        "wr": np.ascontiguousarray(Wr),                                    # [4,128,128]
        "br3": np.ascontiguousarray(br[3:4, :]),                           # [1,128]
        "brt": np.ascontiguousarray(br.T),                                 # [128,4]
        "b0": np.ascontiguousarray(b0.reshape(HID, 1)),                    # [128,1]
        "identbf": np.eye(128, dtype=f).astype(bf),                        # [128,128] bf16
        "onescolbf": np.ones((128, 1), dtype=f).astype(bf),                # [128,1] bf16
        "onesrow": np.ones((1, 128), dtype=f),                             # [1,128] f32
    }
    return [dict(common) for _ in range(NCORES)]


def build_nc(stage=5):
    """stage: debug bisect knob. 5 = full kernel."""
    nc = bacc.Bacc("TRN2", target_bir_lowering=False, debug=False,
                   num_devices=NCORES)
    AF = mybir.ActivationFunctionType
    OP = mybir.AluOpType

    def din(name, shape, dt=F32):
        return nc.dram_tensor(name, shape, dt, kind="ExternalInput").ap()

    t_w0hm = din("w0hm8", [128, 16384], F8)
    t_w0a = din("w0a8", [128, 16384], F8)
    t_w0m = din("w0m", [128, 384], BF16)
    t_xmt = din("xmt", [128, 48], BF16)
    t_adj = din("adj8", [128, 2048], F8)
    t_bet16 = din("bet16", [16, 128], BF16)
    t_bett = din("bett", [128, 16])
    t_lft = din("lft", [256, 272], BF16)
    t_wht = din("wht", [128, 1])
    t_whcol = din("whcol", [128, 1], BF16)
    t_a12 = din("a12", [128, 2])
    t_a3 = din("a3", [128, 1])
    t_wet = din("wet", [128, 17])
    t_we0 = din("we0", [17, 1])
    t_wr = din("wr", [4, 128, 128])
    t_br3 = din("br3", [1, 128])
    t_brt = din("brt", [128, 4])
    t_b0 = din("b0", [128, 1])
    t_ident = din("identbf", [128, 128], BF16)
    t_onescol = din("onescolbf", [128, 1], BF16)
    t_onesrow = din("onesrow", [1, 128])
    t_out = nc.dram_tensor("out", [16, 128], F32, kind="ExternalOutput").ap()

    with tile.TileContext(nc) as tc:
        with tc.tile_pool(name="sb", bufs=1) as sb, \
             tc.tile_pool(name="ps", bufs=1, space="PSUM") as ps, \
             tc.tile_pool(name="dram", bufs=1, space="DRAM") as dram:

            def prog():
                # ---------------------------------------------- batch A DMAs
                def load(name, src, dt=F32, shape=None):
                    t = sb.tile(shape or list(src.shape), dt, tag=name)
                    nc.sync.dma_start(t[:], src)
                    return t

                bett_sb = load("bett", t_bett)
                bet16_sb = load("bet16", t_bet16, BF16)
                wht_sb = load("wht", t_wht)
                whcol_sb = load("whcol", t_whcol, BF16)
                a12_sb = load("a12", t_a12)
                a3_sb = load("a3", t_a3)
                wet_sb = load("wet", t_wet)
                xmt_sb = load("xmt", t_xmt, BF16)
                w0m_sb = load("w0m", t_w0m, BF16)
                ident_sb = load("ident", t_ident, BF16)
                onescol_sb = load("onescol", t_onescol, BF16)
                onesrow_sb = load("onesrow", t_onesrow)
                b0_sb = load("b0", t_b0)
                brT_sb = load("brT", t_brt)
                br3_sb = load("br3", t_br3)
                lft_t = [load(f"lft{h}", t_lft[h * 128:(h + 1) * 128, :], BF16)
                         for h in (0, 1)]
                adj_sb = load("adj", t_adj, F8)
                # betbc[i, b*128+j] = bet[b, j]  (partition-broadcast)
                betbc_sb = sb.tile([128, 2048], BF16, tag="betbc")
                nc.sync.dma_start(
                    betbc_sb[:],
                    t_bet16.rearrange("b j -> (b j)").unsqueeze(0)
                    .broadcast_to((128, 2048)),
                )

                # ---------------------------------------------- tiny weight math
                # [q, k] = WH @ [a1 a2]
                ps_qk = ps.tile([1, 2], F32, tag="small")
                nc.tensor.matmul(ps_qk[:], wht_sb[:], a12_sb[:], start=True, stop=True)
                qk_sb = sb.tile([1, 2], F32, tag="qksb")
                nc.vector.tensor_copy(qk_sb[:], ps_qk[:])

                # a3e[e] = sum_h WE[e,h]*a3[h]
                ps_a3e = ps.tile([17, 1], F32, tag="small")
                nc.tensor.matmul(ps_a3e[:], wet_sb[:], a3_sb[:], start=True, stop=True)
                lhsT2_sb = sb.tile([17, 2], F32, tag="lhsT2")
                nc.vector.tensor_copy(lhsT2_sb[:, 0:1], ps_a3e[:])
                nc.sync.dma_start(lhsT2_sb[:, 1:2], t_we0)

                # link-feature sum over edges -> [1, 272] (free = (e, b))
                ps_lfs = ps.tile([1, 272], F32, tag="small")
                for h in (0, 1):
                    nc.tensor.matmul(ps_lfs[:], onescol_sb[:], lft_t[h][:],
                                     start=(h == 0), stop=(h == 1))
                lfm_sb = sb.tile([1, 272], F32, tag="lfm")
                nc.vector.tensor_copy(lfm_sb[:], ps_lfs[:])
                lfmT_sb = sb.tile([17, 16], F32, tag="lfmT")
                nc.sync.dma_start(
                    lfmT_sb[:],
                    lfm_sb[:].rearrange("p (e b) -> p e b", b=16),
                )

                # s_e = (1/256) * a3e^T @ lfmean^T ; ec0 likewise  -> [1, 16] each
                ps_se = ps.tile([1, 16], F32, tag="small")
                nc.tensor.matmul(ps_se[:], lhsT2_sb[:, 0:1], lfmT_sb[:],
                                 start=True, stop=True)
                se_row = sb.tile([1, 16], F32, tag="serow")
                nc.scalar.activation(se_row[:], ps_se[:], AF.Copy, bias=0.0,
                                     scale=1.0 / M_EDGES)
                ps_ec0 = ps.tile([1, 16], F32, tag="small")
                nc.tensor.matmul(ps_ec0[:], lhsT2_sb[:, 1:2], lfmT_sb[:],
                                 start=True, stop=True)
                ec0_row = sb.tile([1, 16], F32, tag="ec0row")
                nc.scalar.activation(ec0_row[:], ps_ec0[:], AF.Copy, bias=0.0,
                                     scale=1.0 / M_EDGES)

                # se_bc[i,b] = se[b]; ec0_bc[i,b] = ec0[b]  (ones-matmul broadcast)
                ps_sebc = ps.tile([128, 16], F32, tag="bc")
                nc.tensor.matmul(ps_sebc[:], onesrow_sb[:], se_row[:],
                                 start=True, stop=True)
                sebc_sb = sb.tile([128, 16], F32, tag="sebcsb")
                nc.vector.tensor_copy(sebc_sb[:], ps_sebc[:])
                ps_ec0bc = ps.tile([128, 16], F32, tag="bc")
                nc.tensor.matmul(ps_ec0bc[:], onesrow_sb[:], ec0_row[:],
                                 start=True, stop=True)
                ec0bc_sb = sb.tile([128, 16], F32, tag="ec0bcsb")
                nc.vector.tensor_copy(ec0bc_sb[:], ps_ec0bc[:])

                # q_bc / k_bc via DRAM bounce (partition broadcast lives DRAM-side)
                d_qk = dram.tile([1, 2], F32, tag="dqk")
                nc.sync.dma_start(d_qk[:], qk_sb[:])
                q_bc = sb.tile([128, 1], F32, tag="qbc")
                nc.sync.dma_start(q_bc[:], d_qk[0:1, 0:1].broadcast_to((128, 1)))
                k_bc = sb.tile([128, 1], F32, tag="kbc")
                nc.sync.dma_start(k_bc[:], d_qk[0:1, 1:2].broadcast_to((128, 1)))

                # pp[i,b] = q*bet[b,i] + se[b]
                pp_sb = sb.tile([128, 16], F32, tag="pp")
                nc.vector.scalar_tensor_tensor(pp_sb[:], bett_sb[:], q_bc[:],
                                               sebc_sb[:], OP.mult, OP.add)

                neg31_sb = sb.tile([128, 1], F32, tag="neg31")
                nc.vector.memset(neg31_sb[:], -31.0)

                # ---------------------------------------------- gate, then W0 chain
                # Each gate DMA reads a just-DMA'd tile, forcing the SP engine
                # to wait for that tile's completion before posting the next
                # chunk -- so the big chunks complete sequentially and compute
                # pipelines behind the DMA stream.
                gate_n = [0]

                def gate(src_tile, dt):
                    g = dram.tile([1, 4], dt, tag=f"gate{gate_n[0]}")
                    gate_n[0] += 1
                    nc.sync.dma_start(g[:], src_tile[0:1, 0:4])
                gate(betbc_sb, BF16)
                gate(lft_t[1], BF16)
                gate(adj_sb, F8)

                w0hm_c = []
                w0a_c = []
                for c in range(NCHUNK):
                    t = sb.tile([128, 4096], F8, tag=f"w0hm{c}")
                    nc.sync.dma_start(t[:], t_w0hm[:, c * 4096:(c + 1) * 4096])
                    w0hm_c.append(t)
                    gate(t, F8)
                for c in range(NCHUNK):
                    t = sb.tile([128, 4096], F8, tag=f"w0a{c}")
                    nc.sync.dma_start(t[:], t_w0a[:, c * 4096:(c + 1) * 4096])
                    w0a_c.append(t)
                    if c < NCHUNK - 1:
                        gate(t, F8)
                wr_sb = sb.tile([128, 512], F32, tag="wr")
                nc.sync.dma_start(
                    wr_sb[:].rearrange("p (t g) -> p t g", t=4),
                    t_wr.transpose((1, 0, 2)),
                )

                if stage <= 1:
                    dbg = sb.tile([128, 16], F32, tag="dbg1")
                    nc.vector.tensor_copy(dbg[:], pp_sb[:])
                    nc.sync.dma_start(
                        t_out.rearrange("a b -> (a b)").rearrange(
                            "(p f) -> p f", p=128),
                        dbg[:])
                    return

                # ---------------------------------------------- main psum: misc block
                ps_main = ps.tile([128, 16], F32, tag="main")
                for t in range(3):
                    nc.tensor.matmul(ps_main[:],
                                     w0m_sb[:, t * 128:(t + 1) * 128],
                                     xmt_sb[:, t * 16:(t + 1) * 16],
                                     start=(t == 0), stop=False)

                # ---------------------------------------------- G^T build (128 MMs)
                if stage not in (25, 26, 27, 28, 29):
                    ps_gt = ps.tile([128, 128], F32, tag="gt")
                    for i in range(128):
                        ch = w0hm_c[i // 32]
                        nc.tensor.matmul(ps_gt[:, i:i + 1],
                                         ch[:, (i % 32) * 128:(i % 32 + 1) * 128],
                                         whcol_sb[:],
                                         start=True, stop=True)
                    gt_sb = sb.tile([128, 128], BF16, tag="gt")
                    nc.vector.tensor_copy(gt_sb[:], ps_gt[:])
                    ps_g = ps.tile([128, 128], BF16, tag="g")
                    nc.tensor.transpose(ps_g[:], gt_sb[:], ident_sb[:])
                    g_sb = sb.tile([128, 128], BF16, tag="g")
                    nc.vector.tensor_copy(g_sb[:], ps_g[:])

                if stage <= 2:
                    dbg = sb.tile([16, 128], F32, tag="dbg2")
                    nc.vector.tensor_copy(dbg[:], gt_sb[0:16, :])
                    nc.sync.dma_start(t_out, dbg[:])
                    return

                # ---------------------------------------------- attention (16 tiles)
                alpha_all = sb.tile([128, 2048], BF16, tag="alpha")
                wT_sb = sb.tile([128, 16], BF16, tag="wT")
                for b in range(BATCH):
                    bsl = slice(b * 128, (b + 1) * 128)
                    tt = sb.tile([128, 128], BF16, tag=f"tt{b}")
                    nc.scalar.activation(tt[:], betbc_sb[:, bsl], AF.Tanh,
                                         bias=pp_sb[:, b:b + 1], scale=k_bc[:])
                    if stage == 26:
                        nc.vector.tensor_copy(alpha_all[:, bsl], tt[:])
                        continue
                    m01 = sb.tile([128, 128], BF16, tag=f"m01{b}")
                    nc.vector.tensor_scalar(m01[:], adj_sb[:, bsl], 0.0, None,
                                            OP.is_gt)
                    stt = sb.tile([128, 128], BF16, tag=f"stt{b}")
                    nc.vector.scalar_tensor_tensor(stt[:], m01[:], 31.0, tt[:],
                                                   OP.mult, OP.add)
                    if stage == 27:
                        nc.vector.tensor_copy(alpha_all[:, bsl], stt[:])
                        continue
                    un = sb.tile([128, 128], BF16, tag=f"un{b}")
                    rowsum = sb.tile([128, 1], F32, tag=f"rows{b}")
                    nc.scalar.activation(un[:], stt[:], AF.Exp,
                                         bias=neg31_sb[:], scale=1.0,
                                         accum_out=rowsum[:])
                    recip = sb.tile([128, 1], F32, tag=f"recip{b}")
                    nc.vector.reciprocal(recip[:], rowsum[:])
                    nc.vector.tensor_scalar(alpha_all[:, bsl], un[:], recip[:],
                                            None, OP.mult)
                    if stage == 28:
                        continue
                    # w path: multiply on Pool, reduce on Vector, scale on Pool
                    tmp = sb.tile([128, 128], BF16, tag=f"wtmp{b}")
                    nc.gpsimd.tensor_tensor(tmp[:], un[:], betbc_sb[:, bsl],
                                            OP.mult)
                    r_un = sb.tile([128, 1], F32, tag=f"run{b}")
                    nc.vector.reduce_sum(r_un[:], tmp[:],
                                         axis=mybir.AxisListType.X)
                    if stage == 29:
                        continue
                    nc.gpsimd.tensor_scalar(wT_sb[:, b:b + 1], r_un[:], recip[:],
                                            ec0bc_sb[:, b:b + 1], OP.mult,
                                            OP.mult)

                if stage <= 3 or stage in (25, 26, 27, 28, 29):
                    dbg = sb.tile([16, 128], F32, tag="dbg3")
                    nc.vector.tensor_copy(dbg[:], alpha_all[0:16, 0:128])
                    nc.sync.dma_start(t_out, dbg[:])
                    return

                # ---------------------------------------------- Hm block: w @ G
                nc.tensor.matmul(ps_main[:], g_sb[:], wT_sb[:],
                                 start=False, stop=False)

                # ---------------------------------------------- alpha block (128 MMs)
                for j in range(128):
                    ch = w0a_c[j // 32]
                    nc.tensor.matmul(ps_main[:],
                                     ch[:, (j % 32) * 128:(j % 32 + 1) * 128],
                                     alpha_all[:, j:2048:128],
                                     start=False, stop=(j == 127))

                if stage <= 4:
                    nc.sync.dma_start(
                        t_out.rearrange("a b -> (a b)").rearrange(
                            "(p f) -> p f", p=128),
                        ps_main[:])
                    return

                # ---------------------------------------------- bias+relu + MLP
                xT = sb.tile([128, 16], F32, tag="x1T")
                nc.scalar.activation(xT[:], ps_main[:], AF.Relu, bias=b0_sb[:],
                                     scale=1.0)
                for t in range(3):
                    ps_l = ps.tile([128, 16], F32, tag="psl")
                    nc.tensor.matmul(ps_l[:], wr_sb[:, t * 128:(t + 1) * 128],
                                     xT[:], start=True, stop=True)
                    xT_next = sb.tile([128, 16], F32, tag=f"x{t + 2}T")
                    nc.scalar.activation(xT_next[:], ps_l[:], AF.Relu,
                                         bias=brT_sb[:, t:t + 1], scale=1.0)
                    xT = xT_next
                ps_x5 = ps.tile([16, 128], F32, tag="psx5")
                nc.tensor.matmul(ps_x5[:], xT[:], wr_sb[:, 384:512],
                                 start=True, stop=False)
                nc.tensor.matmul(ps_x5[:], onesrow_sb[0:1, 0:16], br3_sb[:],
                                 start=False, stop=True)
                out_sb = sb.tile([16, 128], F32, tag="outsb")
                nc.scalar.activation(out_sb[:], ps_x5[:], AF.Relu, bias=0.0,
                                     scale=1.0)
                nc.sync.dma_start(t_out, out_sb[:])

            prog()
    nc.compile()
    return nc


_compiled_nc = None


def get_nc():
    global _compiled_nc
    if _compiled_nc is None:
        _compiled_nc = build_nc()
    return _compiled_nc


def kernel(**inputs):
    nc = get_nc()
    in_maps = shard_inputs(**inputs)
    res = bass_utils.run_bass_kernel_spmd(nc, in_maps, core_ids=list(range(NCORES)))
    return np.asarray(res.results[0]["out"], dtype=np.float32)


if __name__ == "__main__":
    nc = build_nc()
    print("build + compile OK;", len(nc.main_func.blocks), "blocks")



# revision 4
# speedup vs baseline: 1.6910x; 1.6910x over previous
"""Trainium2 Bass kernel for nn_DeepRMSAFeatureExtractor.

Strategy (8 NeuronCores, SPMD, collective-free, batch-sharded):
  exec time is the MAX per-core first-to-last-instruction span, so launch
  skew between cores is free as long as no core waits on another. Each
  core therefore computes ONLY its 2 of the 16 batch rows end-to-end
  (host concatenates the per-core [128,2] outputs -- pure unshard), while
  the weight stream (W0 fp8, 4.2 MB) is replicated per core because every
  output row needs every W0 element and cross-core traffic would import
  launch skew into the measured span.

  vs the previous replicated-full-batch kernel:
    - batch work (attention tanh/exp/softmax chains, MLP) drops 8x
    - the DMA gate-chain is gone: the profile showed it throttled the
      16 SDMA engines to ~35% busy (~107 GB/s aggregate). Chunks now
      stream back-to-back in FIFO ring order at full rate.
    - ~25 small dma_starts (26.7us of Sync-engine issue time) collapse
      into 3 packed loads (f32 / bf16 / f8 packs built host-side by
      pure relayout+cast).
    - DRAM-bounce broadcasts replaced with PE ones-matmul broadcasts;
      link-feature sums are computed directly as [17,2] on partitions.
  Math identical to before: W0's Hm block collapses via
  x_hm @ W0hm = w @ G with G built on-device from fp8 W0hm; the alpha
  block contracts k-tile j over i with rhs = alpha_all[:, j::128].
"""

import sys

sys.path.insert(0, "/opt/trn_rl_repo")

import numpy as np

import concourse.bass as bass  # noqa: F401  (registers AP machinery)
import concourse.bacc as bacc
import concourse.mybir as mybir
import concourse.tile as tile
from concourse import bass_utils

F32 = mybir.dt.float32
BF16 = mybir.dt.bfloat16
F8 = mybir.dt.float8e4

NCORES = 8
BATCH = 16
BPC = BATCH // NCORES   # batches per core = 2
N = 128                 # nodes == HID
HID = 128
M_EDGES = 256
EF = 17
KP = 5

# offsets into the flat `inputs` row (length 86721)
OFF_SD = 0
OFF_SLOT = 256
OFF_SPEC = 261
OFF_LF = 321
OFF_BET = 4673
OFF_ADJ = 4801

MISC_ROWS = 321                  # source_dest 256 + slots 5 + c_band 30 + l_band 30
HM_OFF = 0
MISC_OFF = N * HID               # 16384
ALPHA_OFF = MISC_OFF + MISC_ROWS  # 16705

NCHUNK = 8                       # w0big split into 8 chunks of 32 k-tiles

# ---- f32 pack column map ----
FC_BETT = 0    # [128, 2]   bet^T for this core's batches
FC_WHT = 2     # [128, 1]   WH^T
FC_A12 = 3     # [128, 2]   [a1 a2]
FC_A3 = 5      # [128, 1]   a3
FC_WET = 6     # [128, 17]  WE^T
FC_B0 = 23     # [128, 1]
FC_BRT = 24    # [128, 4]   br^T
FC_WE0 = 28    # [128, 1]   WE[:,0] padded to 128 rows
FC_WR = 29     # [128, 512] Wr[t] stacked: [p, t*128+h]
NF32 = 541

# ---- bf16 pack column map ----
BC_WHCOL = 0    # [128, 1]
BC_XMT = 1      # [128, 6]    misc features: [p, t*2+r]
BC_W0M = 7      # [128, 384]  W0 misc block: [p, t*128+h]
BC_IDENT = 391  # [128, 128]  identity
BC_LFT = 519    # [128, 68]   link features: [p, h*34 + r*17 + e]
BC_BETBC = 587  # [128, 256]  bet broadcast: [i, r*128+j]
NBF16 = 843

NF8 = BPC * N   # adj pack [i, r*128+j]


def shard_inputs(inputs, WH, WE, a_attn, W0, b0, Wr, br):
    """Host-side prep: slicing / transposition / padding / dtype cast only."""
    f = np.float32
    bf = mybir.dt.np(BF16)
    f8 = mybir.dt.np(F8)
    X = np.asarray(inputs, dtype=f)
    WH = np.asarray(WH, dtype=f).reshape(1, HID)
    WE = np.asarray(WE, dtype=f)
    a_attn = np.asarray(a_attn, dtype=f)
    W0 = np.asarray(W0, dtype=f)
    b0 = np.asarray(b0, dtype=f)
    Wr = np.asarray(Wr, dtype=f)
    br = np.asarray(br, dtype=f)

    # ---- shared (weight) sections ----
    # w0hm8[f, i*128 + h] = W0[i*128 + f, h]
    w0hm8 = (
        W0[HM_OFF:HM_OFF + N * HID].reshape(N, HID, HID)
        .transpose(1, 0, 2).reshape(128, N * HID)
    )
    # w0a8[i, j*128 + h] = W0[ALPHA_OFF + i*128 + j, h]
    w0a8 = W0[ALPHA_OFF:ALPHA_OFF + N * N].reshape(128, N * HID)
    w0big = np.ascontiguousarray(
        np.concatenate([w0hm8, w0a8], axis=1)).astype(f8)  # [128, 32768]

    w0m = np.zeros((3 * 128, HID), dtype=f)
    w0m[:MISC_ROWS] = W0[MISC_OFF:MISC_OFF + MISC_ROWS]
    w0m_dev = w0m.reshape(3, 128, HID).transpose(1, 0, 2).reshape(128, 3 * HID)

    f32_shared = np.zeros((128, NF32), dtype=f)
    f32_shared[:, FC_WHT] = WH[0]
    f32_shared[:, FC_A12 + 0] = a_attn[:HID, 0]
    f32_shared[:, FC_A12 + 1] = a_attn[HID:2 * HID, 0]
    f32_shared[:, FC_A3] = a_attn[2 * HID:, 0]
    # WE is [17, 128]; wet[h, e] = WE[e, h] -> rows h (128), cols e (17)
    f32_shared[:, FC_WET:FC_WET + EF] = WE.T
    f32_shared[:, FC_B0] = b0
    f32_shared[:, FC_BRT:FC_BRT + 4] = br.T
    f32_shared[:EF, FC_WE0] = WE[:, 0]
    f32_shared[:, FC_WR:FC_WR + 512] = Wr.transpose(1, 0, 2).reshape(128, 512)

    bf16_shared = np.zeros((128, NBF16), dtype=f)
    bf16_shared[:, BC_WHCOL] = WH[0]
    bf16_shared[:, BC_W0M:BC_W0M + 384] = w0m_dev
    bf16_shared[:, BC_IDENT:BC_IDENT + 128] = np.eye(128, dtype=f)

    # ---- per-core (batch-sharded) sections ----
    in_maps = []
    for c in range(NCORES):
        bsel = slice(c * BPC, (c + 1) * BPC)
        Xb = X[bsel]                                           # [2, 86721]
        bet = Xb[:, OFF_BET:OFF_BET + N]                       # [2, 128]
        adj = Xb[:, OFF_ADJ:OFF_ADJ + N * N].reshape(BPC, N, N)
        lf = Xb[:, OFF_LF:OFF_LF + M_EDGES * EF].reshape(BPC, M_EDGES, EF)
        spec = Xb[:, OFF_SPEC:OFF_SPEC + KP * 12].reshape(BPC, KP, 2, 6)

        xm = np.zeros((BPC, 3 * 128), dtype=f)
        xm[:, 0:256] = Xb[:, OFF_SD:OFF_SD + 256]
        xm[:, 256:261] = Xb[:, OFF_SLOT:OFF_SLOT + KP]
        xm[:, 261:291] = spec[:, :, 0, :].reshape(BPC, 30)
        xm[:, 291:321] = spec[:, :, 1, :].reshape(BPC, 30)

        f32p = f32_shared.copy()
        f32p[:, FC_BETT:FC_BETT + BPC] = bet.T

        bf16p = bf16_shared.copy()
        # xmt[p, t*2 + r] = xm[r, t*128 + p]
        bf16p[:, BC_XMT:BC_XMT + 3 * BPC] = (
            xm.T.reshape(3, 128, BPC).transpose(1, 0, 2).reshape(128, 3 * BPC)
        )
        # lft[p, h*34 + r*17 + e] = lf[r, h*128 + p, e]
        bf16p[:, BC_LFT:BC_LFT + 2 * BPC * EF] = (
            lf.transpose(1, 0, 2).reshape(2, 128, BPC * EF)
            .transpose(1, 0, 2).reshape(128, 2 * BPC * EF)
        )
        # betbc[i, r*128 + j] = bet[r, j]
        bf16p[:, BC_BETBC:BC_BETBC + BPC * N] = np.broadcast_to(
            bet.reshape(1, BPC * N), (128, BPC * N))

        f8p = adj.transpose(1, 0, 2).reshape(128, BPC * N)     # [i, r*128+j]

        in_maps.append({
            "f32p": np.ascontiguousarray(f32p),
            "bf16p": np.ascontiguousarray(bf16p).astype(bf),
            "f8p": np.ascontiguousarray(f8p).astype(f8),
            "w0big": w0big,
        })
    return in_maps


def build_nc():
    nc = bacc.Bacc("TRN2", target_bir_lowering=False, debug=False,
                   num_devices=NCORES)
    AF = mybir.ActivationFunctionType
    OP = mybir.AluOpType

    t_f32p = nc.dram_tensor("f32p", [128, NF32], F32, kind="ExternalInput").ap()
    t_bf16p = nc.dram_tensor("bf16p", [128, NBF16], BF16, kind="ExternalInput").ap()
    t_f8p = nc.dram_tensor("f8p", [128, NF8], F8, kind="ExternalInput").ap()
    t_w0big = nc.dram_tensor("w0big", [128, 32768], F8, kind="ExternalInput").ap()
    t_out = nc.dram_tensor("out", [128, BPC], F32, kind="ExternalOutput").ap()

    with tile.TileContext(nc) as tc:
        with tc.tile_pool(name="sb", bufs=1) as sb, \
             tc.tile_pool(name="ps", bufs=1, space="PSUM") as ps:

            # -------------------------------------------- DMA issue (FIFO ring)
            f32v = sb.tile([128, NF32], F32, tag="f32p")
            nc.sync.dma_start(f32v[:], t_f32p)
            bf16v = sb.tile([128, NBF16], BF16, tag="bf16p")
            nc.sync.dma_start(bf16v[:], t_bf16p)
            adj_sb = sb.tile([128, NF8], F8, tag="f8p")
            nc.sync.dma_start(adj_sb[:], t_f8p)
            w0c = []
            for ci in range(NCHUNK):
                t = sb.tile([128, 4096], F8, tag=f"w0c{ci}")
                nc.sync.dma_start(t[:], t_w0big[:, ci * 4096:(ci + 1) * 4096])
                w0c.append(t)

            # views into the packs
            bett = f32v[:, FC_BETT:FC_BETT + BPC]
            wht = f32v[:, FC_WHT:FC_WHT + 1]
            a12 = f32v[:, FC_A12:FC_A12 + 2]
            a3 = f32v[:, FC_A3:FC_A3 + 1]
            wet = f32v[:, FC_WET:FC_WET + EF]
            b0v = f32v[:, FC_B0:FC_B0 + 1]
            brT = f32v[:, FC_BRT:FC_BRT + 4]
            we0 = f32v[:, FC_WE0:FC_WE0 + 1]
            wrv = f32v[:, FC_WR:FC_WR + 512]
            whcol = bf16v[:, BC_WHCOL:BC_WHCOL + 1]
            xmt = bf16v[:, BC_XMT:BC_XMT + 3 * BPC]
            w0m = bf16v[:, BC_W0M:BC_W0M + 384]
            ident = bf16v[:, BC_IDENT:BC_IDENT + 128]
            lft = bf16v[:, BC_LFT:BC_LFT + 2 * BPC * EF]
            betbc = bf16v[:, BC_BETBC:BC_BETBC + BPC * N]

            # -------------------------------------------- constants (no DMA)
            onesrow = sb.tile([1, 128], F32, tag="onesrow")
            nc.vector.memset(onesrow[:], 1.0)
            onescol = sb.tile([128, 1], BF16, tag="onescol")
            nc.vector.memset(onescol[:], 1.0)
            neg31 = sb.tile([128, 1], F32, tag="neg31")
            nc.vector.memset(neg31[:], -31.0)

            # -------------------------------------------- tiny weight math
            # [q, k] = WH @ [a1 a2]; broadcast to all partitions via ones-matmul
            ps_qk = ps.tile([1, 2], F32, tag="small")
            nc.tensor.matmul(ps_qk[:], wht, a12, start=True, stop=True)
            qk_sb = sb.tile([1, 2], F32, tag="qksb")
            nc.vector.tensor_copy(qk_sb[:], ps_qk[:])
            ps_qkbc = ps.tile([128, 2], F32, tag="bc")
            nc.tensor.matmul(ps_qkbc[:], onesrow[:], qk_sb[:], start=True, stop=True)
            qkbc = sb.tile([128, 2], F32, tag="qkbc")
            nc.vector.tensor_copy(qkbc[:], ps_qkbc[:])
            q_bc = qkbc[:, 0:1]
            k_bc = qkbc[:, 1:2]

            # a3e[e] = sum_h WE[e,h]*a3[h];  lhsT2 = [a3e, WE[:,0]]  [17,2]
            ps_a3e = ps.tile([17, 1], F32, tag="small")
            nc.tensor.matmul(ps_a3e[:], wet, a3, start=True, stop=True)
            lhsT2 = sb.tile([17, 2], F32, tag="lhsT2")
            nc.vector.tensor_copy(lhsT2[:, 0:1], ps_a3e[:])
            nc.vector.tensor_copy(lhsT2[:, 1:2], we0[0:EF, :])

            # link-feature sums on partitions: lfmT[e, r] = sum_m lf[r, m, e]
            ps_lf = ps.tile([EF, BPC], F32, tag="small")
            for r in range(BPC):
                for h in range(2):
                    nc.tensor.matmul(
                        ps_lf[:, r:r + 1],
                        lft[:, h * 34 + r * EF:h * 34 + (r + 1) * EF],
                        onescol[:], start=(h == 0), stop=(h == 1))
            lfmT = sb.tile([EF, BPC], F32, tag="lfmT")
            nc.vector.tensor_copy(lfmT[:], ps_lf[:])

            # seec_row = [se_0, se_1, ec0_0, ec0_1] / 256 ; broadcast to [128,4]
            ps_seec = ps.tile([1, 2 * BPC], F32, tag="small")
            nc.tensor.matmul(ps_seec[:, 0:BPC], lhsT2[:, 0:1], lfmT[:],
                             start=True, stop=True)
            nc.tensor.matmul(ps_seec[:, BPC:2 * BPC], lhsT2[:, 1:2], lfmT[:],
                             start=True, stop=True)
            seec_row = sb.tile([1, 2 * BPC], F32, tag="seecrow")
            nc.scalar.activation(seec_row[:], ps_seec[:], AF.Copy, bias=0.0,
                                 scale=1.0 / M_EDGES)
            ps_seecbc = ps.tile([128, 2 * BPC], F32, tag="bc")
            nc.tensor.matmul(ps_seecbc[:], onesrow[:], seec_row[:],
                             start=True, stop=True)
            seecbc = sb.tile([128, 2 * BPC], F32, tag="seecbc")
            nc.vector.tensor_copy(seecbc[:], ps_seecbc[:])
            sebc = seecbc[:, 0:BPC]

            # pp[i,r] = q*bet[r,i] + se[r]
            pp = sb.tile([128, BPC], F32, tag="pp")
            nc.vector.scalar_tensor_tensor(pp[:], bett, q_bc, sebc,
                                           OP.mult, OP.add)

            # -------------------------------------------- attention (BPC tiles)
            alpha_all = sb.tile([128, BPC * N], BF16, tag="alpha")
            wT_sb = sb.tile([128, BPC], BF16, tag="wT")
            for r in range(BPC):
                bsl = slice(r * 128, (r + 1) * 128)
                tt = sb.tile([128, 128], BF16, tag=f"tt{r}")
                nc.scalar.activation(tt[:], betbc[:, bsl], AF.Tanh,
                                     bias=pp[:, r:r + 1], scale=k_bc)
                m01 = sb.tile([128, 128], BF16, tag=f"m01{r}")
                nc.vector.tensor_scalar(m01[:], adj_sb[:, bsl], 0.0, None,
                                        OP.is_gt)
                stt = sb.tile([128, 128], BF16, tag=f"stt{r}")
                nc.vector.scalar_tensor_tensor(stt[:], m01[:], 31.0, tt[:],
                                               OP.mult, OP.add)
                un = sb.tile([128, 128], BF16, tag=f"un{r}")
                rowsum = sb.tile([128, 1], F32, tag=f"rows{r}")
                nc.scalar.activation(un[:], stt[:], AF.Exp,
                                     bias=neg31[:], scale=1.0,
                                     accum_out=rowsum[:])
                recip = sb.tile([128, 1], F32, tag=f"recip{r}")
                nc.vector.reciprocal(recip[:], rowsum[:])
                nc.vector.tensor_scalar(alpha_all[:, bsl], un[:], recip[:],
                                        None, OP.mult)
                tmp = sb.tile([128, 128], BF16, tag=f"wtmp{r}")
                nc.gpsimd.tensor_tensor(tmp[:], un[:], betbc[:, bsl], OP.mult)
                r_un = sb.tile([128, 1], F32, tag=f"run{r}")
                nc.vector.reduce_sum(r_un[:], tmp[:], axis=mybir.AxisListType.X)
                nc.gpsimd.tensor_scalar(wT_sb[:, r:r + 1], r_un[:], recip[:],
                                        seecbc[:, BPC + r:BPC + r + 1],
                                        OP.mult, OP.mult)

            # -------------------------------------------- misc block into main
            ps_main = ps.tile([128, BPC], F32, tag="main")
            for t in range(3):
                nc.tensor.matmul(ps_main[:],
                                 w0m[:, t * 128:(t + 1) * 128],
                                 xmt[:, t * BPC:(t + 1) * BPC],
                                 start=(t == 0), stop=False)

            # -------------------------------------------- G^T build (128 MMs)
            ps_gt = ps.tile([128, 128], F32, tag="gt")
            for i in range(128):
                ch = w0c[i // 32]
                nc.tensor.matmul(ps_gt[:, i:i + 1],
                                 ch[:, (i % 32) * 128:(i % 32 + 1) * 128],
                                 whcol, start=True, stop=True)
            gt_sb = sb.tile([128, 128], BF16, tag="gtsb")
            nc.vector.tensor_copy(gt_sb[:], ps_gt[:])
            ps_g = ps.tile([128, 128], BF16, tag="g")
            nc.tensor.transpose(ps_g[:], gt_sb[:], ident)
            g_sb = sb.tile([128, 128], BF16, tag="gsb")
            nc.vector.tensor_copy(g_sb[:], ps_g[:])

            # -------------------------------------------- Hm block: w @ G
            nc.tensor.matmul(ps_main[:], g_sb[:], wT_sb[:],
                             start=False, stop=False)

            # -------------------------------------------- alpha block (128 MMs)
            for j in range(128):
                ch = w0c[4 + j // 32]
                nc.tensor.matmul(ps_main[:],
                                 ch[:, (j % 32) * 128:(j % 32 + 1) * 128],
                                 alpha_all[:, j:BPC * N:128],
                                 start=False, stop=(j == 127))

            # -------------------------------------------- bias+relu + MLP
            xT = sb.tile([128, BPC], F32, tag="x1T")
            nc.scalar.activation(xT[:], ps_main[:], AF.Relu, bias=b0v, scale=1.0)
            for t in range(4):
                ps_l = ps.tile([128, BPC], F32, tag="psl")
                nc.tensor.matmul(ps_l[:], wrv[:, t * 128:(t + 1) * 128],
                                 xT[:], start=True, stop=True)
                xT_next = sb.tile([128, BPC], F32, tag=f"x{t + 2}T")
                nc.scalar.activation(xT_next[:], ps_l[:], AF.Relu,
                                     bias=brT[:, t:t + 1], scale=1.0)
                xT = xT_next
            nc.sync.dma_start(t_out, xT[:])
    nc.compile()
    return nc


_compiled_nc = None


def get_nc():
    global _compiled_nc
    if _compiled_nc is None:
        _compiled_nc = build_nc()
    return _compiled_nc


def gather(results):
    """[128, 2] per core -> [16, 128] full output (pure unshard)."""
    return np.concatenate(
        [np.asarray(results[c]["out"], dtype=np.float32).T
         for c in range(NCORES)], axis=0)


def kernel(**inputs):
    nc = get_nc()
    in_maps = shard_inputs(**inputs)
    res = bass_utils.run_bass_kernel_spmd(nc, in_maps, core_ids=list(range(NCORES)))
    return gather(res.results)


if __name__ == "__main__":
    nc = build_nc()
    print("build + compile OK;", len(nc.main_func.blocks), "blocks")
